# revision 42
# baseline (speedup 1.0000x reference)
"""Trainium2 Bass kernel: single transformer block (MHA + FFN + 2xLN).

Sharding: data-parallel over tokens. 8 cores; cores 0-3 own batch 0,
cores 4-7 own batch 1; each core owns 1024 consecutive tokens of its
batch. QKV/FFN/LN are token-local; attention needs all K/V of the
batch, obtained with 3 pipelined combined K+V AllGathers over each
4-core group.

v3 layout strategy: weights/activations are bf16 (PSUM stays fp32)
except the attention inner loop, which runs fp8e4m3 with DoubleRow
matmuls: scores contract dk=64 as [32,2]-packed operands (2x), and
ctx contracts kv pairs of 128-chunks as [128,2]-packed (4x, since the
65-col V tile only fills half the PE otherwise). V tiles are padded
to 96 columns (multiple-of-32 rule for dual-fp8 LDWEIGHTS) with a
ones column at 64 producing the softmax denominator in-psum. The exp
(Act engine, fp32 PSUM -> fp8 SBUF) is the attention bottleneck
(~428us); the PE has large slack there, which absorbs the wo/w1/w2
weight transposes interleaved as fill work. fp8 error injected in
attention is damped by the 1/sqrt(d) wo scaling and the residual
(x + mha) structure; measured end-to-end error stays ~0.5%.
LayerNorm statistics use ones-vector matmuls on the PE; softmax and
LN reciprocals use DVE reciprocal_approx_fast off the critical path.
"""

import os
import sys

for _p in (
    "/opt/trn_rl_repo",
    "/root/.axon_site",
    "/root/.axon_site/_ro/trn_rl_repo",
    "/root/.axon_site/_ro/pypackages",
):
    if os.path.isdir(_p) and _p not in sys.path:
        sys.path.append(_p)

import numpy as np

import concourse.bass as bass
import concourse.mybir as mybir
import concourse.tile as tile
from concourse import bacc
from concourse.bass_utils import run_bass_kernel_spmd
from concourse.masks import make_identity

F32 = mybir.dt.float32
F32R = mybir.dt.float32r
BF = mybir.dt.bfloat16
F8 = mybir.dt.float8e4
AF = mybir.ActivationFunctionType
ALU = mybir.AluOpType
DR = mybir.MatmulPerfMode.DoubleRow

B, S, D = 2, 4096, 768
H, DK = 12, 64
DFF = 3072
NCORES = 8
GROUP = 4  # cores per batch
TOK = (B * S) // NCORES  # 1024 tokens per core
TCH = TOK // 128  # 8
DCH = D // 128  # 6
FCH = DFF // 128  # 24
KV = S  # kv length per batch
KCH = KV // 128  # 32
EPS = 1e-5
RG = [[0, 1, 2, 3], [4, 5, 6, 7]]

NG = 3  # pipelined sub-gathers (4 heads each)
HPG = H // NG  # heads per sub-gather (4)
CPG = HPG // 2  # K.T 128-row chunks per sub-gather (2)
VW = 96  # V columns per head: 64 value cols, ones col at 64, zero pad
KG_ELEMS = 128 * CPG * TOK  # fp8 elems of K.T per sub-gather
VG_ELEMS = TCH * 128 * (HPG * VW)  # fp8 elems of V per sub-gather


def _percol(tc, const, t_in, name, n):
    """1D [n*128] fp32 -> SBUF [128, n] (feature-chunked per-column)."""
    nc = tc.nc
    t = const.tile([128, n], F32, tag=f"pc_{name}", name=f"pc_{name}")
    nc.sync.dma_start(t[:], t_in[name].rearrange("(c p) -> p c", p=128))
    return t


def _emit_ln(tc, ps_bc, ps_st, sb_tmp, y, g_sb, beta_sb, out):
    """LayerNorm along the partition (feature) axis of y [128, DCH, TOK] bf16.

    Stats via PE ones-matmuls into a single [33, TOK] PSUM tile (row 0 =
    sum, row 32 = sum of squares), sqrt on Act + approx reciprocal on DVE,
    broadcasts via fp32 ones-column matmuls, apply via DVE + Act.
    """
    nc = tc.nc
    ones_p = tc._ones_p_bf
    ones_f = tc._ones_f32
    st = ps_st.tile([33, TOK], F32, tag="st", name="st")
    for q in range(TOK // 512):
        qs = slice(q * 512, (q + 1) * 512)
        for j in range(DCH):
            nc.tensor.matmul(
                st[0:1, qs], ones_p[:], y[:, j, qs],
                start=(j == 0), stop=(j == DCH - 1), skip_group_check=True,
            )
    for j in range(DCH):
        sq = sb_tmp.tile([128, TOK], BF, tag="lnsq", name="sq")
        nc.vector.tensor_tensor(sq[:], y[:, j, :], y[:, j, :], ALU.mult)
        for q in range(TOK // 512):
            qs = slice(q * 512, (q + 1) * 512)
            nc.tensor.matmul(
                st[32:33, qs], ones_p[:], sq[:, qs],
                start=(j == 0), stop=(j == DCH - 1), skip_group_check=True,
            )
    mu = sb_tmp.tile([1, TOK], F32, tag="lnmu", name="mu")
    var = sb_tmp.tile([1, TOK], F32, tag="lnvar", name="var")
    rs = sb_tmp.tile([1, TOK], F32, tag="lnrs", name="rs")
    brow = sb_tmp.tile([1, TOK], F32, tag="lnbrow", name="brow")
    mu2 = sb_tmp.tile([1, TOK], F32, tag="lnmu2", name="mu2")
    nc.vector.tensor_scalar_mul(mu[:], st[0:1, :], 1.0 / D)
    nc.vector.tensor_scalar_mul(var[:], st[32:33, :], 1.0 / D)
    nc.vector.tensor_tensor(mu2[:], mu[:], mu[:], ALU.mult)  # mu^2
    nc.vector.tensor_tensor(var[:], var[:], mu2[:], ALU.subtract)
    # sd = sqrt(var + eps) on Act, then rs = 1/sd on DVE (approx is fine
    # at this tolerance)
    nc.scalar.activation(var[:], var[:], AF.Sqrt, bias=tc._eps[:])
    nc.vector.reciprocal_approx_fast(rs[:], var[:])
    nc.vector.tensor_tensor(brow[:], mu[:], rs[:], ALU.mult)  # mu*rs
    bcA = ps_bc.tile([128, TOK], F32, tag="big", name="bcA")
    bcB = ps_bc.tile([128, TOK], F32, tag="big", name="bcB")
    for q in range(TOK // 512):
        qs = slice(q * 512, (q + 1) * 512)
        nc.tensor.matmul(bcA[:, qs], ones_f[:], rs[:, qs],
                         start=True, stop=True, skip_group_check=True)
        nc.tensor.matmul(bcB[:, qs], ones_f[:], brow[:, qs],
                         start=True, stop=True, skip_group_check=True)
    for j in range(DCH):
        t1 = sb_tmp.tile([128, TOK], F32, tag="lnt", name="t1")
        nc.vector.tensor_tensor(t1[:], y[:, j, :], bcA[:], ALU.mult)
        nc.vector.tensor_tensor(t1[:], t1[:], bcB[:], ALU.subtract)
        nc.scalar.activation(out[:, j, :], t1[:], AF.Identity,
                             bias=beta_sb[:, j : j + 1], scale=g_sb[:, j : j + 1])


def _emit_body(tc, t_in, t_out):
    nc = tc.nc
    dbg = {k[4:]: v for k, v in t_out.items() if k.startswith("dbg_")}

    def dump(name, sb_ap):
        if name in dbg:
            nc.sync.dma_start(dbg[name], sb_ap)

    x_ap = t_in["x_shard"]
    out_ap = t_out["out_shard"]

    from contextlib import ExitStack

    with tc.tile_pool(name="const", bufs=1) as const, \
         tc.tile_pool(name="dram", bufs=1, space="DRAM") as dram, \
         tc.tile_pool(name="pAct", bufs=1) as pAct:
        _pw_stack = ExitStack()
        pW = _pw_stack.enter_context(tc.tile_pool(name="pW", bufs=1))

        ident = const.tile([128, 128], F32)
        make_identity(nc, ident[:])
        ident_bf = const.tile([128, 128], BF)
        nc.vector.tensor_copy(ident_bf[:], ident[:])
        ones_bf_col = const.tile([128, 1], BF)
        nc.vector.memset(ones_bf_col[:], 1.0)
        ones_bf_row = const.tile([1, 128], BF)
        nc.vector.memset(ones_bf_row[:], 1.0)
        ones_f32 = const.tile([1, 128], F32)
        nc.vector.memset(ones_f32[:], 1.0)
        ones_bf_h = const.tile([128, H], BF)
        nc.vector.memset(ones_bf_h[:], 1.0)
        ones_bf_tok = const.tile([1, TOK], BF)
        nc.vector.memset(ones_bf_tok[:], 1.0)
        eps_sb = const.tile([1, 1], F32)
        nc.vector.memset(eps_sb[:], EPS)
        negc_sb = const.tile([128, 1], F32)
        nc.vector.memset(negc_sb[:], -4.0)
        tc._ones_p_bf = ones_bf_col
        tc._ones_f32 = ones_f32
        tc._eps = eps_sb

        bq_sb = _percol(tc, const, t_in, "bq", DCH)
        bk_sb = _percol(tc, const, t_in, "bk", DCH)
        bo_sb = _percol(tc, const, t_in, "bo", DCH)
        b1_sb = _percol(tc, const, t_in, "b1", FCH)
        b2_sb = _percol(tc, const, t_in, "b2", DCH)
        g1_sb = _percol(tc, const, t_in, "g1", DCH)
        beta1_sb = _percol(tc, const, t_in, "beta1", DCH)
        g2_sb = _percol(tc, const, t_in, "g2", DCH)
        beta2_sb = _percol(tc, const, t_in, "beta2", DCH)
        bv_row32 = const.tile([1, D], F32)
        nc.sync.dma_start(bv_row32[:], t_in["bv"].unsqueeze(0))
        bv_row = const.tile([1, D], BF)
        nc.vector.tensor_copy(bv_row[:], bv_row32[:])
        bq_row32 = const.tile([1, D], F32)
        nc.sync.dma_start(bq_row32[:], t_in["bq"].unsqueeze(0))
        bq_row = const.tile([1, D], BF)
        nc.vector.tensor_copy(bq_row[:], bq_row32[:])

        # DRAM scratch for the combined K+V all-gathers (fp8)
        kv_ins = [dram.tile([KG_ELEMS + VG_ELEMS], F8, tag=f"kvi{g}",
                            name=f"kv_in{g}") for g in range(NG)]
        kv_outs = [dram.tile([GROUP, KG_ELEMS + VG_ELEMS], F8, tag=f"kvo{g}",
                             name=f"kv_out{g}") for g in range(NG)]

        # Big activation tiles (bf16), reused across phases via tags.
        xT = pAct.tile([128, DCH, TOK], BF, tag="slotA")    # A..C (residual 1)
        QT = pAct.tile([128, DCH, TOK], BF, tag="slotQ")  # A..B
        woT = pW.tile([128, DCH, D], BF, tag="woT")         # filled in B, used C
        w1T = pW.tile([128, DCH, DFF], BF, tag="w1T")       # filled in B, used D
        w2T = pW.tile([128, FCH, D], BF, tag="w2T")         # filled in B, used D

        # =================== Phases A..C ===================================
        if True:

            # ---- Phase A: x transpose, V, K, Q projections, gathers -------
            with tc.tile_pool(name="pA", bufs=2) as pA, \
                 tc.tile_pool(name="pA1", bufs=3) as pA1, \
                 tc.tile_pool(name="ps_tp", bufs=2, space="PSUM") as ps_tp, \
                 tc.tile_pool(name="ps_qk", bufs=2, space="PSUM") as ps_qk:

                def transpose_w_bf(w_ap, dest, n_out_ch, n_in_ch):
                    """dest[:, j, i*128:(i+1)*128] = w[i, j].T (bf16 out)."""
                    for i in range(n_out_ch):
                        win = pA1.tile([128, n_in_ch * 128], F32, tag="win",
                                       name="win")
                        nc.sync.dma_start(
                            win[:], w_ap[i * 128 : (i + 1) * 128, :])
                        for j in range(n_in_ch):
                            tp = ps_tp.tile([128, 128], F32, tag="tp", name="tp")
                            nc.tensor.transpose(
                                tp[:], win[:, j * 128 : (j + 1) * 128], ident[:])
                            nc.vector.tensor_copy(
                                dest[:, j, i * 128 : (i + 1) * 128], tp[:])

                # x -> xT (bf16)
                for t in range(TCH):
                    xin = pA1.tile([128, D], F32, tag="xin", name="xin")
                    nc.sync.dma_start(xin[:], x_ap[t * 128 : (t + 1) * 128, :])
                    for j in range(DCH):
                        tp = ps_tp.tile([128, 128], F32, tag="tp", name="tp")
                        nc.tensor.transpose(tp[:], xin[:, j * 128 : (j + 1) * 128],
                                            ident[:])
                        nc.vector.tensor_copy(xT[:, j, t * 128 : (t + 1) * 128],
                                              tp[:])

                # wv/wk/wq transposed weights rotate through 2 shared buffers
                wT = {}

                def next_wT(wname):
                    wT[wname] = pA.tile([128, DCH, D], BF, tag="wT",
                                        name=f"{wname}T")

                # V (natural layout [tok, dout]; per-head 96-wide fp8 block:
                # 64 value cols, ones col, zero pad)
                next_wT("wv")
                transpose_w_bf(t_in["wv"], wT["wv"], DCH, DCH)
                for t in range(TCH):
                    psv = ps_qk.tile([128, TOK], F32, tag="qk", name="psv")
                    for lo, hi in ((0, 512), (512, D)):
                        qs = slice(lo, hi)
                        for j in range(DCH):
                            nc.tensor.matmul(
                                psv[:, qs],
                                xT[:, j, t * 128 : (t + 1) * 128],
                                wT["wv"][:, j, qs],
                                start=(j == 0), stop=False,
                                skip_group_check=True,
                            )
                        nc.tensor.matmul(  # bias row: + ones.T @ bv
                            psv[:, qs], ones_bf_row[:], bv_row[0:1, qs],
                            start=False, stop=True, skip_group_check=True,
                        )
                    vt = pA1.tile([128, H * VW], F8, tag="vtev", name="vt")
                    vt_h = vt[:].rearrange("p (h f) -> p h f", h=H)
                    nc.vector.memset(vt_h[:, :, DK + 1 : VW], 0.0)
                    nc.vector.tensor_copy(
                        vt_h[:, :, 0:DK],
                        psv[:, 0:D].rearrange("p (h f) -> p h f", h=H),
                    )
                    nc.vector.tensor_copy(
                        vt_h[:, :, DK : DK + 1], ones_bf_h[:].unsqueeze(2),
                    )
                    for g in range(NG):
                        nc.gpsimd.dma_start(
                            kv_ins[g][KG_ELEMS:].rearrange(
                                "(t p f) -> t p f", t=TCH, p=128)[t],
                            vt[:, g * HPG * VW : (g + 1) * HPG * VW],
                        )

                # K: fp8 K.T chunks; launch combined sub-gather g after its
                # chunk pair (V part of the region is already written).
                next_wT("wk")
                transpose_w_bf(t_in["wk"], wT["wk"], DCH, DCH)
                for m in range(DCH):
                    pso = ps_qk.tile([128, TOK], F32, tag="qk", name="pso")
                    for q in range(TOK // 512):
                        qs = slice(q * 512, (q + 1) * 512)
                        for j in range(DCH):
                            nc.tensor.matmul(
                                pso[:, qs],
                                wT["wk"][:, j, m * 128 : (m + 1) * 128],
                                xT[:, j, qs],
                                start=(j == 0), stop=(j == DCH - 1),
                                skip_group_check=True,
                            )
                    kt = pA1.tile([128, TOK], F8, tag="ktev", name="kt")
                    nc.scalar.activation(kt[:], pso[:], AF.Identity,
                                         bias=bk_sb[:, m : m + 1])
                    g = m // CPG
                    nc.gpsimd.dma_start(
                        kv_ins[g][0:KG_ELEMS].rearrange(
                            "(p c t) -> p c t", p=128, c=CPG)[:, m % CPG, :],
                        kt[:],
                    )
                    if m % CPG == CPG - 1:
                        nc.gpsimd.collective_compute(
                            "AllGather", ALU.bypass, replica_groups=RG,
                            ins=[kv_ins[g][:].opt()],
                            outs=[kv_outs[g][:].opt()],
                        )

                # Q: bf16 QT via Act bias writes
                next_wT("wq")
                transpose_w_bf(t_in["wq"], wT["wq"], DCH, DCH)
                for m in range(DCH):
                    pso = ps_qk.tile([128, TOK], F32, tag="qk", name="pso")
                    for q in range(TOK // 512):
                        qs = slice(q * 512, (q + 1) * 512)
                        for j in range(DCH):
                            nc.tensor.matmul(
                                pso[:, qs],
                                wT["wq"][:, j, m * 128 : (m + 1) * 128],
                                xT[:, j, qs],
                                start=(j == 0), stop=(j == DCH - 1),
                                skip_group_check=True,
                            )
                    nc.scalar.activation(QT[:, m, :], pso[:], AF.Identity,
                                         bias=bq_sb[:, m : m + 1])
                dump("xT", xT[:])

            # ---- Phase B: attention over head pairs -----------------------
            # Both heads of a pair live on partition halves 0-63 / 64-127;
            # their contract-64 bf16 score matmuls are interleaved so the PE
            # runs them concurrently on disjoint row-halves (2x). ctx stays
            # fp8 DoubleRow (2 kv chunks per instruction).
            ctxT = pAct.tile([128, DCH, TOK], BF, tag="slotC")  # B..C

            with tc.tile_pool(name="pB", bufs=2) as pB, \
                 tc.tile_pool(name="pK8", bufs=1) as pK8, \
                 tc.tile_pool(name="pBe", bufs=4) as pBe, \
                 tc.tile_pool(name="pBt", bufs=1) as pBt, \
                 tc.tile_pool(name="ps_sc", bufs=2, space="PSUM") as ps_sc, \
                 tc.tile_pool(name="ps_ce", bufs=1, space="PSUM") as ps_ce, \
                 tc.tile_pool(name="ps_co", bufs=1, space="PSUM") as ps_co:

                for jch in range(DCH):  # head pair (2*jch, 2*jch+1)
                    g = (2 * jch) // HPG
                    cc = jch % CPG
                    K8f = pK8.tile([128, KV], F8, tag="k8", name="K8f")
                    KhT = pB.tile([128, KV], BF, tag="kh", name="KhT")
                    Vhe = pB.tile([128, KCH, VW], F8, tag="vhe", name="Vhe")
                    Vho = pB.tile([128, KCH, VW], F8, tag="vho", name="Vho")
                    for r in range(GROUP):
                        nc.sync.dma_start(
                            K8f[:, r * TOK : (r + 1) * TOK],
                            kv_outs[g][r, 0:KG_ELEMS].rearrange(
                                "(p c t) -> p c t", p=128, c=CPG)[:, cc, :],
                        )
                        for hh, Vt in (((2 * jch) % HPG, Vhe),
                                       ((2 * jch + 1) % HPG, Vho)):
                            nc.sync.dma_start(
                                Vt[:, r * TCH : (r + 1) * TCH, :],
                                kv_outs[g][r, KG_ELEMS:].rearrange(
                                    "(t p f) -> p t f", t=TCH, p=128
                                )[:, :, hh * VW : (hh + 1) * VW],
                            )
                    nc.vector.tensor_copy(KhT[:], K8f[:])  # fp8 -> bf16
                    acc_e = ps_ce.tile([VW, TOK], F32, tag="ce", name="acc_e")
                    acc_o = ps_co.tile([VW, TOK], F32, tag="co", name="acc_o")
                    Ee_pair = Eo_pair = Ee_prev = Eo_prev = None
                    for c in range(KCH):
                        ps_se = ps_sc.tile([128, TOK], F32, tag="s",
                                           name="ps_se")
                        ps_so = ps_sc.tile([128, TOK], F32, tag="s",
                                           name="ps_so")
                        for q in range(TOK // 512):
                            qs = slice(q * 512, (q + 1) * 512)
                            nc.tensor.matmul(
                                ps_se[:, qs],
                                KhT[0:64, c * 128 : (c + 1) * 128],
                                QT[0:64, jch, qs],
                                start=True, stop=True, skip_group_check=True,
                            )
                            nc.tensor.matmul(
                                ps_so[:, qs],
                                KhT[64:128, c * 128 : (c + 1) * 128],
                                QT[64:128, jch, qs],
                                start=True, stop=True, skip_group_check=True,
                            )
                        if c % 2 == 0:
                            Ee_pair = pBe.tile([128, 2, TOK], F8, tag="E",
                                               name="Ee")
                            Eo_pair = pBe.tile([128, 2, TOK], F8, tag="E",
                                               name="Eo")
                        # bias -4.0 keeps exp outputs within fp8e4m3 range
                        # (cancels exactly in the softmax normalization)
                        nc.scalar.activation(Ee_pair[:, c % 2, :], ps_se[:],
                                             AF.Exp, bias=negc_sb[:],
                                             scale=1.0 / float(np.sqrt(DK)))
                        nc.scalar.activation(Eo_pair[:, c % 2, :], ps_so[:],
                                             AF.Exp, bias=negc_sb[:],
                                             scale=1.0 / float(np.sqrt(DK)))
                        if c % 2 == 0 and c >= 2:
                            pc = c - 2
                            for q in range(TOK // 512):
                                qs = slice(q * 512, (q + 1) * 512)
                                nc.tensor.matmul(
                                    acc_e[:, qs], Vhe[:, pc : pc + 2, :],
                                    Ee_prev[:, :, qs],
                                    start=(pc == 0), stop=False,
                                    skip_group_check=True, perf_mode=DR,
                                )
                                nc.tensor.matmul(
                                    acc_o[:, qs], Vho[:, pc : pc + 2, :],
                                    Eo_prev[:, :, qs],
                                    start=(pc == 0), stop=False,
                                    skip_group_check=True, perf_mode=DR,
                                )
                        if c % 2 == 1:
                            Ee_prev, Eo_prev = Ee_pair, Eo_pair
                    for q in range(TOK // 512):
                        qs = slice(q * 512, (q + 1) * 512)
                        nc.tensor.matmul(
                            acc_e[:, qs], Vhe[:, KCH - 2 : KCH, :],
                            Ee_prev[:, :, qs], start=False, stop=True,
                            skip_group_check=True, perf_mode=DR,
                        )
                        nc.tensor.matmul(
                            acc_o[:, qs], Vho[:, KCH - 2 : KCH, :],
                            Eo_prev[:, :, qs], start=False, stop=True,
                            skip_group_check=True, perf_mode=DR,
                        )
                    # normalize both heads off the critical path
                    for plo, acc in ((0, acc_e), (64, acc_o)):
                        den = pBt.tile([1, TOK], F32, tag="den", name="den")
                        nc.vector.tensor_copy(den[:], acc[64:65, :])
                        rec = pBt.tile([1, TOK], F32, tag="rec", name="rec")
                        nc.vector.reciprocal_approx_fast(rec[:], den[:])
                        bcr = pBt.tile([64, TOK], F32, tag="bcr", name="bcr")
                        nc.gpsimd.partition_broadcast(bcr[:], rec[:])
                        nc.vector.tensor_tensor(
                            ctxT[plo : plo + 64, jch, :], acc[0:64, :],
                            bcr[:], ALU.mult,
                        )

            # ---- wo/w1/w2 transposes (in-order PE: must precede C/D) ------
            with tc.tile_pool(name="pWp", bufs=6) as pWp, \
                 tc.tile_pool(name="ps_tpw", bufs=4, space="PSUM") as ps_tpw:

                def wprep(w_ap, n_out_ch, dest_fn, col0=0):
                    for i in range(n_out_ch):
                        win = pWp.tile([128, DCH * 128], F32, tag="win",
                                       name="win")
                        nc.sync.dma_start(
                            win[:], w_ap[i * 128 : (i + 1) * 128,
                                         col0 : col0 + DCH * 128])
                        for j in range(DCH):
                            tp = ps_tpw.tile([128, 128], F32, tag="tp",
                                             name="tp")
                            nc.tensor.transpose(
                                tp[:], win[:, j * 128 : (j + 1) * 128],
                                ident[:])
                            nc.vector.tensor_copy(dest_fn(i, j), tp[:])

                wprep(t_in["wo"], DCH,
                      lambda i, j: woT[:, j, i * 128 : (i + 1) * 128])
                wprep(t_in["w1"], FCH,
                      lambda i, j: w1T[:, j, i * 128 : (i + 1) * 128])
                for quarter in range(4):
                    wprep(t_in["w2"], DCH,
                          lambda i, j, _q=quarter: w2T[:, _q * DCH + j,
                                                       i * 128 : (i + 1) * 128],
                          col0=quarter * D)

            dump("ctxT", ctxT[:])

            # ---- Phase C: O-projection + residual + LN1 -------------------
            n1 = pAct.tile([128, DCH, TOK], BF, tag="slotB")

            with tc.tile_pool(name="pC2", bufs=2) as pC2, \
                 tc.tile_pool(name="ps_o", bufs=2, space="PSUM") as ps_o, \
                 tc.tile_pool(name="ps_st", bufs=1, space="PSUM") as ps_st:
                y1 = pAct.tile([128, DCH, TOK], BF, tag="slotD", name="y1")
                for m in range(DCH):
                    pso = ps_o.tile([128, TOK], F32, tag="big", name="pso")
                    for q in range(TOK // 512):
                        qs = slice(q * 512, (q + 1) * 512)
                        for j in range(DCH):
                            nc.tensor.matmul(
                                pso[:, qs],
                                woT[:, j, m * 128 : (m + 1) * 128],
                                ctxT[:, j, qs],
                                start=(j == 0), stop=(j == DCH - 1),
                                skip_group_check=True,
                            )
                    # y1 = (pso + bo) + x  (fused on DVE)
                    nc.vector.scalar_tensor_tensor(
                        y1[:, m, :], pso[:], bo_sb[:, m : m + 1], xT[:, m, :],
                        ALU.add, ALU.add,
                    )
                dump("y1", y1[:])
                _emit_ln(tc, ps_o, ps_st, pC2, y1, g1_sb, beta1_sb, n1)
                dump("n1", n1[:])

        # =================== Phase D: FFN (+ residual) =====================
        y2 = pAct.tile([128, DCH, TOK], BF, tag="slotA")  # reuses xT slot
        with tc.tile_pool(name="ps_f2", bufs=1, space="PSUM") as ps_f2, \
             tc.tile_pool(name="ps_h", bufs=2, space="PSUM") as ps_h, \
             tc.tile_pool(name="pDh", bufs=3) as pDh:
            for half in range(2):
                hs = slice(half * 512, (half + 1) * 512)
                ps2 = ps_f2.tile([128, DCH, 512], F32, tag="ffn2", name="ps2")

                def emit_psh(i):
                    psh = ps_h.tile([128, 512], F32, tag="h", name="psh")
                    for j in range(DCH):
                        nc.tensor.matmul(
                            psh[:], w1T[:, j, i * 128 : (i + 1) * 128],
                            n1[:, j, hs],
                            start=(j == 0), stop=(j == DCH - 1),
                            skip_group_check=True,
                        )
                    return psh

                # Software-pipelined: psh(i+1) is emitted before ps2(i) so
                # the PE works while Gelu(i) runs on Act.
                psh = emit_psh(0)
                for i in range(FCH):
                    hsb = pDh.tile([128, 512], BF, tag="hsb", name="hsb")
                    nc.scalar.activation(hsb[:], psh[:], AF.Gelu,
                                         bias=b1_sb[:, i : i + 1])
                    if i + 1 < FCH:
                        psh = emit_psh(i + 1)
                    for m in range(DCH):
                        nc.tensor.matmul(
                            ps2[:, m, :], w2T[:, i, m * 128 : (m + 1) * 128],
                            hsb[:],
                            start=(i == 0), stop=(i == FCH - 1),
                            skip_group_check=True,
                        )
                for m in range(DCH):
                    nc.vector.scalar_tensor_tensor(
                        y2[:, m, hs], ps2[:, m, :], b2_sb[:, m : m + 1],
                        n1[:, m, hs], ALU.add, ALU.add,
                    )
        dump("y2", y2[:])
        _pw_stack.close()  # free woT/w1T/w2T before phase E

        # =================== Phase E: LN2 + output transpose ===============
        yf = pAct.tile([128, DCH, TOK], BF, tag="slotC")  # reuses ctxT slot
        with tc.tile_pool(name="pE2", bufs=2) as pE2, \
             tc.tile_pool(name="ps_bc2", bufs=2, space="PSUM") as ps_bc2:
            with tc.tile_pool(name="ps_st2", bufs=1, space="PSUM") as ps_st2:
                _emit_ln(tc, ps_bc2, ps_st2, pE2, y2, g2_sb, beta2_sb, yf)
            with tc.tile_pool(name="ps_tp2", bufs=2, space="PSUM") as ps_tp2:
                for t in range(TCH):
                    on = pE2.tile([128, D], F32, tag="on", name="on")
                    for j in range(DCH):
                        tp = ps_tp2.tile([128, 128], BF, tag="tp2", name="tp")
                        nc.tensor.transpose(tp[:],
                                            yf[:, j, t * 128 : (t + 1) * 128],
                                            ident_bf[:])
                        nc.vector.tensor_copy(on[:, j * 128 : (j + 1) * 128],
                                              tp[:])
                    nc.sync.dma_start(out_ap[t * 128 : (t + 1) * 128, :], on[:])


_CACHE = {}

DBG_SPECS = {
    "xT": ([128, DCH, TOK], BF), "Q8": ([128, H // 2, 2, TOK], F8),
    "K80": ([32, 2, KV], F8), "Vh0": ([128, KCH, VW], F8),
    "E0": ([128, TOK], F8), "S0": ([128, TOK], F32),
    "ctxT": ([128, DCH, TOK], BF), "y1": ([128, DCH, TOK], BF),
    "n1": ([128, DCH, TOK], BF), "y2": ([128, DCH, TOK], BF),
}


def _build():
    if "nc" in _CACHE:
        return _CACHE["nc"]
    debug = os.environ.get("KERNEL_DEBUG", "0") == "1"
    nc = bacc.Bacc("TRN2", target_bir_lowering=False, debug=False,
                   num_devices=NCORES)
    t_in = {}
    t_in["x_shard"] = nc.dram_tensor("x_shard", [TOK, D], F32,
                                     kind="ExternalInput").ap()
    for name, shape in (
        ("wq", [D, D]), ("bq", [D]), ("wk", [D, D]), ("bk", [D]),
        ("wv", [D, D]), ("bv", [D]), ("wo", [D, D]), ("bo", [D]),
        ("w1", [DFF, D]), ("b1", [DFF]), ("w2", [D, DFF]), ("b2", [D]),
        ("g1", [D]), ("beta1", [D]), ("g2", [D]), ("beta2", [D]),
    ):
        t_in[name] = nc.dram_tensor(name, shape, F32, kind="ExternalInput").ap()
    t_out = {"out_shard": nc.dram_tensor("out_shard", [TOK, D], F32,
                                         kind="ExternalOutput").ap()}
    if debug:
        for name, (shape, dt) in DBG_SPECS.items():
            t_out["dbg_" + name] = nc.dram_tensor(
                "dbg_" + name, shape, dt, kind="ExternalOutput").ap()
    with tile.TileContext(nc) as tc:
        _emit_body(tc, t_in, t_out)
    nc.compile()
    _CACHE["nc"] = nc
    return nc


def _in_maps(inputs):
    f = lambda k: np.ascontiguousarray(np.asarray(inputs[k], dtype=np.float32))
    x = f("x")
    shared = {k: f(k) for k in inputs if k != "x"}
    maps = []
    for core in range(NCORES):
        g, r = divmod(core, GROUP)
        m = dict(shared)
        m["x_shard"] = np.ascontiguousarray(x[g, r * TOK : (r + 1) * TOK, :])
        maps.append(m)
    return maps


def kernel(**inputs):
    nc = _build()
    maps = _in_maps(inputs)
    res = run_bass_kernel_spmd(nc, maps, core_ids=list(range(NCORES)))
    shards = [res.results[i]["out_shard"] for i in range(NCORES)]
    out = np.concatenate(shards, axis=0).reshape(B, S, D)
    return out.astype(np.float32)
